# revision 2
# baseline (speedup 1.0000x reference)
"""Trainium2 Bass kernel for nn_MoEBlock — fp8-DoubleRow MoE, bf16 attention,
chunk-pipelined (attention chunk ch overlaps MoE chunk ch-1 on other engines).

Sharding: heads 2/16 per core (attention), expert e on core e (MoE).
x/8 is injected into the attention AllReduce so ar_out == x1 directly.
z AllReduce (f32) writes straight into outT.
"""

import sys

if "/opt/trn_rl_repo" not in sys.path:
    sys.path.insert(0, "/opt/trn_rl_repo")

import ml_dtypes
import numpy as np

import concourse.bacc as bacc
import concourse.mybir as mybir
import concourse.tile as tile
from concourse.bass_utils import run_bass_kernel_spmd
from concourse.masks import make_identity

B, S, D, H, F, E, K = 2, 2048, 1024, 16, 4096, 8, 2
HD = D // H          # 64
T = B * S            # 4096
EPS = 1e-6
N_CORES = 8
HPC = H // N_CORES   # 2 heads per core
HCOL = HPC * HD      # 128

P = 128
QC = 512             # token tile (matmul free dim)
CW = 1024            # chunk width (tokens); 4 chunks
NCH = T // CW        # 4
NKT = S // P         # 16 k-tiles per batch
NDC = D // P         # 8
NFC = F // P         # 32
SC = 32.0            # fp8 weight scale

f32 = mybir.dt.float32
f32r = mybir.dt.float32r
bf16 = mybir.dt.bfloat16
f8 = mybir.dt.float8e4
AX = mybir.AxisListType
ALU = mybir.AluOpType
ACT = mybir.ActivationFunctionType
DR = mybir.MatmulPerfMode.DoubleRow

_NC_CACHE = {}


def build_nc(sim_mode=False, taps=False):
    key = (sim_mode, taps)
    if key in _NC_CACHE:
        return _NC_CACHE[key]
    nc = bacc.Bacc("TRN2", target_bir_lowering=False, debug=False,
                   num_devices=1 if sim_mode else N_CORES)

    def all_reduce(src_ap, dst_ap):
        if sim_mode:
            nc.sync.dma_start(dst_ap[0:1, :], src_ap[0:1, :])
        else:
            nc.gpsimd.collective_compute(
                "AllReduce", ALU.add,
                replica_groups=[list(range(N_CORES))],
                ins=[src_ap], outs=[dst_ap],
            )

    # ---- I/O ----
    xT = nc.dram_tensor("xT", [D, T], f32, kind="ExternalInput")
    xTb = nc.dram_tensor("xTb", [D, T], bf16, kind="ExternalInput")
    wq = nc.dram_tensor("wq", [P, NDC, HCOL], bf16, kind="ExternalInput")
    wk = nc.dram_tensor("wk", [P, NDC, HCOL], bf16, kind="ExternalInput")
    wv = nc.dram_tensor("wv", [P, NDC, HCOL], bf16, kind="ExternalInput")
    wo = nc.dram_tensor("wo", [HCOL, D], bf16, kind="ExternalInput")
    rw = nc.dram_tensor("rw", [P, NDC, E], bf16, kind="ExternalInput")
    w1d = nc.dram_tensor("w1d", [P, NFC, 4, 2, P], mybir.dt.uint8,
                         kind="ExternalInput")
    w2d = nc.dram_tensor("w2d", [P, NDC, 16, 2, P], mybir.dt.uint8,
                         kind="ExternalInput")
    w1l = nc.dram_tensor("w1l", [P, NFC, 4, 2, P], mybir.dt.uint8,
                         kind="ExternalInput")
    w2l = nc.dram_tensor("w2l", [P, NDC, 16, 2, P], mybir.dt.uint8,
                         kind="ExternalInput")
    b1 = nc.dram_tensor("b1", [NFC, P], f32, kind="ExternalInput")
    b2 = nc.dram_tensor("b2", [NDC, P], f32, kind="ExternalInput")
    esel = nc.dram_tensor("esel", [1, E], bf16, kind="ExternalInput")
    outT = nc.dram_tensor("outT", [D, T], f32, kind="ExternalOutput")
    if taps:
        tap_x1 = nc.dram_tensor("tap_x1", [D, T], bf16, kind="ExternalOutput")
        tap_z = nc.dram_tensor("tap_z", [D, T], f32, kind="ExternalOutput")

    with tile.TileContext(nc) as tc:
        with (
            tc.tile_pool(name="const", bufs=1) as cp,
            tc.tile_pool(name="dram", bufs=1, space="DRAM") as dp,
        ):
            # ---- constants / weights (resident) ----
            ident = cp.tile([P, P], f32, tag="ident")
            make_identity(nc, ident[:])
            ident_bf = cp.tile([P, P], bf16, tag="ident_bf")
            nc.vector.tensor_copy(ident_bf[:], ident[:])
            ones_bf = cp.tile([P, 1], bf16, tag="ones_bf")
            nc.gpsimd.memset(ones_bf[:], 1.0)
            ones_f8 = cp.tile([P, 1], f8, tag="ones_f8")
            nc.gpsimd.memset(ones_f8[:], 1.0)
            ones_row = cp.tile([1, P], bf16, tag="ones_row")
            nc.gpsimd.memset(ones_row[:], 1.0)
            eps_col = cp.tile([P, 1], f32, tag="eps_col")
            nc.gpsimd.memset(eps_col[:], EPS)
            masks = cp.tile([P, QC + 3 * P], bf16, tag="masks")
            nc.gpsimd.memset(masks[:], 1.0)
            nc.gpsimd.affine_select(
                out=masks[:], in_=masks[:],
                compare_op=ALU.is_ge, fill=0.0, base=-3 * P,
                pattern=[[1, QC + 3 * P]], channel_multiplier=-1,
            )
            b1_sb = cp.tile([P, NFC], f32, tag="b1_sb")
            nc.sync.dma_start(b1_sb[:], b1[:].rearrange("a p -> p a"))
            b2x_sb = cp.tile([P, NDC], f32, tag="b2x_sb")
            nc.sync.dma_start(b2x_sb[:], b2[:].rearrange("a p -> p a"))
            esel_bc = cp.tile([P, E], bf16, tag="esel_bc")
            nc.sync.dma_start(esel_bc[:], esel[0:1, :].to_broadcast((P, E)))

            wq_sb = cp.tile([P, NDC, HCOL], bf16, tag="wq_sb")
            wk_sb = cp.tile([P, NDC, HCOL], bf16, tag="wk_sb")
            wv_sb = cp.tile([P, NDC, HCOL], bf16, tag="wv_sb")
            wo_sb = cp.tile([P, D], bf16, tag="wo_sb")
            rw_sb = cp.tile([P, NDC, E], bf16, tag="rw_sb")
            for w_sb, w_dr in ((wq_sb, wq), (wk_sb, wk), (wv_sb, wv)):
                nc.sync.dma_start(w_sb[:], w_dr[:])
            nc.sync.dma_start(wo_sb[:], wo[:])
            nc.sync.dma_start(rw_sb[:], rw[:])
            w1sb = cp.tile([P, NFC, 4, 2, P], f8, tag="w1sb")
            w2sb = cp.tile([P, NDC, 16, 2, P], f8, tag="w2sb")

            # ---- DRAM scratch ----
            ar_in = [dp.tile([D, CW], bf16, tag=f"ar_in{i}", name=f"ar_in{i}")
                     for i in range(NCH)]
            ar_out = [dp.tile([D, CW], bf16, tag=f"ar_out{i}", name=f"ar_out{i}",
                              addr_space="Shared") for i in range(NCH)]
            z_in = [dp.tile([D, CW], f32, tag=f"z_in{i}", name=f"z_in{i}")
                    for i in range(NCH)]
            z_out = [dp.tile([D, CW], f32, tag=f"z_out{i}", name=f"z_out{i}",
                             addr_space="Shared") for i in range(NCH)]

            with (
                # psum pools
                tc.tile_pool(name="PS", bufs=1, space="PSUM") as PS,
                # persistent/rotating sbuf pools
                tc.tile_pool(name="ab", bufs=1) as ab,     # per-batch attn
                tc.tile_pool(name="sc", bufs=2) as scp,    # rotating scratch
                tc.tile_pool(name="big", bufs=1) as big,   # single-buffer big
                tc.tile_pool(name="rows", bufs=2) as rp,   # [1,*] rows
            ):
                qT = kT = v_aug = None
                pend = {}

                def attn_phase(ch, qT, kT, v_aug):
                    b_ = ch // 2
                    hf = ch % 2
                    ctok = ch * CW

                    # ---- projections (2 x 512 tokens) ----
                    for tch in range(2):
                        tok = ctok + tch * QC
                        tsl = slice(tok, tok + QC)
                        csl = slice(hf * CW + tch * QC,
                                    hf * CW + (tch + 1) * QC)
                        q_ps = PS.tile([P, QC], f32, tag="ps", bufs=4)
                        k_ps = PS.tile([P, QC], f32, tag="ps", bufs=4)
                        v_ps = PS.tile([P, QC], f32, tag="ps", bufs=4)
                        ssp = PS.tile([P, QC], f32, tag="ps", bufs=4)
                        for h4 in range(2):
                            xtb = scp.tile([P, 4, QC], bf16, tag="xtb")
                            nc.scalar.dma_start(
                                xtb[:],
                                xTb[4 * h4 * P:(4 * h4 + 4) * P, tsl]
                                .rearrange("(a p) t -> p a t", p=P))
                            sq = scp.tile([P, 4, QC], f8, tag="sq")
                            nc.vector.tensor_mul(sq[:], xtb[:], xtb[:])
                            for d4 in range(4):
                                dc = 4 * h4 + d4
                                st = (dc == 0)
                                sp = (dc == NDC - 1)
                                xd = xtb[:, d4, :]
                                nc.tensor.matmul(q_ps[:], wq_sb[:, dc, :], xd,
                                                 start=st, stop=sp)
                                nc.tensor.matmul(k_ps[:], wk_sb[:, dc, :], xd,
                                                 start=st, stop=sp)
                                nc.tensor.matmul(v_ps[:], wv_sb[:, dc, :], xd,
                                                 start=st, stop=sp)
                                nc.tensor.matmul(
                                    ssp[0:1, :], ones_f8[:, 0:1], sq[:, d4, :],
                                    start=st, stop=sp)
                        msr = rp.tile([1, QC], f32, tag="rowf")
                        nc.vector.tensor_scalar(msr[:], ssp[0:1, :], 1.0 / D,
                                                EPS, op0=ALU.mult, op1=ALU.add)
                        srr = rp.tile([1, QC], f32, tag="rowf")
                        nc.scalar.sqrt(srr[:], msr[:])
                        r1r = rp.tile([1, QC], bf16, tag="rowb")
                        with nc.allow_low_precision(reason="bf16 rsqrt"):
                            nc.vector.reciprocal(r1r[:], srr[:])
                        r1bp = PS.tile([P, QC], f32, tag="ps", bufs=4)
                        nc.tensor.matmul(r1bp[:], ones_row[0:1, :], r1r[:],
                                         start=True, stop=True)
                        r1s = scp.tile([P, QC], bf16, tag="r1s")
                        nc.vector.tensor_copy(r1s[:], r1bp[:])
                        nc.vector.tensor_mul(qT[:, csl], q_ps[:], r1s[:])
                        nc.vector.tensor_mul(kT[:, csl], k_ps[:], r1s[:])
                        vts = scp.tile([P, QC], bf16, tag="vts", bufs=1)
                        nc.vector.tensor_mul(vts[:], v_ps[:], r1s[:])
                        for blk in range(QC // P):
                            kt_ = hf * 8 + tch * 4 + blk
                            vtp = PS.tile([P, P], bf16, tag="tpb", bufs=1)
                            nc.tensor.transpose(
                                vtp[:], vts[:, blk * P:(blk + 1) * P],
                                ident_bf[:])
                            for h in range(HPC):
                                idx = (h * NKT + kt_) * 65
                                nc.vector.tensor_copy(
                                    v_aug[:, idx:idx + HD],
                                    vtp[:, h * HD:(h + 1) * HD])

                    # ---- scores / softmax / ctx ----
                    ctxn = big.tile([P, CW], bf16, tag="ctxn")
                    for qcl in range(2):
                        qg = hf * 2 + qcl
                        qsl = slice(qg * QC, (qg + 1) * QC)
                        nkt = (qg + 1) * (QC // P)
                        for h in range(HPC):
                            hsl = slice(h * HD, (h + 1) * HD)
                            cps = PS.tile([65, QC], f32, tag="cps", bufs=2)
                            for kt_ in range(nkt):
                                ksl = slice(kt_ * P, (kt_ + 1) * P)
                                sps = PS.tile([P, QC], f32, tag="ps", bufs=4)
                                nc.tensor.matmul(
                                    sps[:], kT[hsl, ksl], qT[hsl, qsl],
                                    start=True, stop=True)
                                ex = scp.tile([P, QC], bf16, tag="ex", bufs=2)
                                nc.scalar.activation(ex[:], sps[:], ACT.Exp)
                                j = kt_ - qg * (QC // P)
                                if j >= 0:
                                    nc.vector.tensor_mul(
                                        ex[:], ex[:],
                                        masks[:, (3 - j) * P:
                                              (3 - j) * P + QC])
                                idx = (h * NKT + kt_) * 65
                                nc.tensor.matmul(
                                    cps[:], v_aug[:, idx:idx + 65], ex[:],
                                    start=(kt_ == 0), stop=(kt_ == nkt - 1))
                            rec = rp.tile([1, QC], bf16, tag="rowb")
                            with nc.allow_low_precision(reason="softmax recip"):
                                nc.vector.reciprocal(rec[:], cps[64:65, :])
                            bc = PS.tile([P, QC], f32, tag="ps", bufs=4)
                            nc.tensor.matmul(bc[0:HD, :],
                                             ones_row[0:1, 0:HD], rec[:],
                                             start=True, stop=True)
                            bcs = scp.tile([HD, QC], bf16, tag="bcs")
                            nc.vector.tensor_copy(bcs[:], bc[0:HD, :])
                            nc.vector.tensor_mul(
                                ctxn[hsl, qcl * QC:(qcl + 1) * QC],
                                cps[0:HD, :], bcs[:])

                    # ---- wo + x/8 inject + AR ----
                    for t in range(2):
                        tsl = slice(ctok + t * QC, ctok + (t + 1) * QC)
                        for dc in range(NDC):
                            wop = PS.tile([P, QC], f32, tag="ps", bufs=4)
                            nc.tensor.matmul(
                                wop[:], wo_sb[:, dc * P:(dc + 1) * P],
                                ctxn[:, t * QC:(t + 1) * QC],
                                start=True, stop=True)
                            xq = scp.tile([P, QC], f32, tag="xq")
                            nc.sync.dma_start(
                                xq[:], xT[dc * P:(dc + 1) * P, tsl])
                            ot = scp.tile([P, QC], bf16, tag="ot")
                            nc.vector.scalar_tensor_tensor(
                                ot[:], xq[:], 1.0 / N_CORES, wop[:],
                                op0=ALU.mult, op1=ALU.add)
                            nc.sync.dma_start(
                                ar_in[ch][dc * P:(dc + 1) * P,
                                          t * QC:(t + 1) * QC], ot[:])
                    all_reduce(ar_in[ch][:], ar_out[ch][:])
                    x1c = []
                    for t in range(2):
                        x1h = big.tile([P, NDC, QC], bf16, tag="x1c", bufs=2,
                                       name=f"x1h{t}")
                        nc.sync.dma_start(
                            x1h[:],
                            ar_out[ch][:, t * QC:(t + 1) * QC].rearrange(
                                "(a p) t -> p a t", p=P))
                        x1c.append(x1h)
                    return x1c

                def tail_a(ch, x1c):
                    ctok = ch * CW
                    # ---- r2, router logits ----
                    lgc = scp.tile([E, CW], f32, tag="lgc", bufs=1)
                    r2bc = scp.tile([P, CW], bf16, tag="r2bc", bufs=1)
                    for t in range(2):
                        tsl = slice(t * QC, (t + 1) * QC)
                        ss2 = PS.tile([P, QC], f32, tag="ps", bufs=4)
                        lgp = PS.tile([P, QC], f32, tag="ps", bufs=4)
                        sq2 = None
                        for dc in range(NDC):
                            st = (dc == 0)
                            sp = (dc == NDC - 1)
                            if dc % 4 == 0:
                                sq2 = scp.tile([P, 4, QC], f8, tag="sq",
                                               bufs=2, name="sq2")
                                nc.vector.tensor_mul(
                                    sq2[:], x1c[t][:, dc:dc + 4, :],
                                    x1c[t][:, dc:dc + 4, :])
                            nc.tensor.matmul(
                                ss2[0:1, :], ones_f8[:, 0:1],
                                sq2[:, dc % 4, :], start=st, stop=sp)
                            nc.tensor.matmul(
                                lgp[0:E, :], rw_sb[:, dc, :],
                                x1c[t][:, dc, :], start=st, stop=sp)
                        nc.vector.tensor_copy(lgc[:, tsl], lgp[0:E, :])
                        msr2 = rp.tile([1, QC], f32, tag="rowf")
                        nc.vector.tensor_scalar(msr2[:], ss2[0:1, :], 1.0 / D,
                                                EPS, op0=ALU.mult, op1=ALU.add)
                        srr2 = rp.tile([1, QC], f32, tag="rowf")
                        nc.scalar.sqrt(srr2[:], msr2[:])
                        r2r = rp.tile([1, QC], bf16, tag="rowb")
                        with nc.allow_low_precision(reason="bf16 rsqrt"):
                            nc.vector.reciprocal(r2r[:], srr2[:])
                        r2bp = PS.tile([P, QC], f32, tag="ps", bufs=4)
                        nc.tensor.matmul(r2bp[:], ones_row[0:1, :], r2r[:],
                                         start=True, stop=True)
                        nc.vector.tensor_copy(
                            r2bc[:, t * QC:(t + 1) * QC], r2bp[:])

                    h2q = big.tile([P, 4, 2, CW], f8, tag="h2q", bufs=1)
                    for t in range(2):
                        for dc in range(NDC):
                            nc.vector.tensor_mul(
                                h2q[:, dc // 2, dc % 2,
                                    t * QC:(t + 1) * QC],
                                x1c[t][:, dc, :],
                                r2bc[:, t * QC:(t + 1) * QC])
                    # ---- r2 per token-tile columns (negated) ----
                    tpg = PS.tile([P, CW // P * E], f32, tag="tps", bufs=1)
                    for i in range(CW // P):
                        nc.tensor.matmul(
                            tpg[:, i:i + 1],
                            r2bc[0:1, i * P:(i + 1) * P], ones_bf[0:1, 0:1],
                            start=True, stop=True)
                    r2col = scp.tile([P, E], f32, tag="r2col", bufs=1)
                    nc.vector.tensor_copy(r2col[:], tpg[:, 0:E])

                    # ---- gates (per token-tile, [P,1]-scalar ops) ----
                    ltp = PS.tile([P, CW // P * E], f32, tag="tps", bufs=1)
                    for tt in range(CW // P):
                        nc.tensor.transpose(
                            ltp[:, tt * E:(tt + 1) * E],
                            lgc[:, tt * P:(tt + 1) * P], ident[0:E, 0:E])
                    gcols = scp.tile([P, CW // P], bf16, tag="gcols", bufs=1)
                    for tt in range(CW // P):
                        lg = scp.tile([P, E], f32, tag="lg", bufs=3)
                        nc.vector.tensor_copy(lg[:], ltp[:, tt * E:(tt + 1) * E])
                        m1 = scp.tile([P, 1], f32, tag="m1", bufs=3)
                        nc.vector.tensor_reduce(m1[:], lg[:], axis=AX.X,
                                                op=ALU.max)
                        mk1 = scp.tile([P, E], f32, tag="mk1", bufs=3)
                        nc.vector.tensor_scalar(mk1[:], lg[:], m1[:], None,
                                                op0=ALU.is_equal)
                        msk = scp.tile([P, E], f32, tag="msk", bufs=3)
                        nc.vector.scalar_tensor_tensor(
                            msk[:], mk1[:], -1e30, lg[:],
                            op0=ALU.mult, op1=ALU.add)
                        m2 = scp.tile([P, 1], f32, tag="m2", bufs=3)
                        nc.vector.tensor_reduce(m2[:], msk[:], axis=AX.X,
                                                op=ALU.max)
                        mk2 = scp.tile([P, E], f32, tag="mk2", bufs=3)
                        nc.vector.tensor_scalar(mk2[:], msk[:], m2[:], None,
                                                op0=ALU.is_equal)
                        dlt = scp.tile([P, 1], f32, tag="dlt", bufs=3)
                        nc.vector.tensor_sub(dlt[:], m1[:], m2[:])
                        g1 = scp.tile([P, 1], f32, tag="g1", bufs=3)
                        nc.scalar.activation(g1[:], dlt[:], ACT.Sigmoid,
                                             scale=r2col[:, tt:tt + 1])
                        g2_ = scp.tile([P, 1], f32, tag="g2_", bufs=3)
                        nc.vector.tensor_scalar(g2_[:], g1[:], -1.0, 1.0,
                                                op0=ALU.mult, op1=ALU.add)
                        gts = scp.tile([P, E], f32, tag="gts", bufs=3)
                        nc.vector.tensor_scalar(gts[:], mk1[:], g1[:], None,
                                                op0=ALU.mult)
                        nc.vector.scalar_tensor_tensor(
                            gts[:], mk2[:], g2_[:], gts[:],
                            op0=ALU.mult, op1=ALU.add)
                        gsel = scp.tile([P, E], bf16, tag="gsel", bufs=3)
                        nc.vector.tensor_mul(gsel[:], gts[:], esel_bc[:])
                        with nc.allow_low_precision(reason="bf16 gate col"):
                            nc.vector.tensor_reduce(gcols[:, tt:tt + 1],
                                                    gsel[:], axis=AX.X,
                                                    op=ALU.add)
                    gbc = scp.tile([P, CW], bf16, tag="gbc", bufs=1)
                    for gh in range(2):
                        grow_ps = PS.tile([P, QC], f32, tag="ps", bufs=4)
                        for tq in range(QC // P):
                            tt = gh * (QC // P) + tq
                            nc.tensor.matmul(
                                grow_ps[0:1, tq * P:(tq + 1) * P],
                                gcols[:, tt:tt + 1], ident_bf[:],
                                start=True, stop=True)
                        grow = rp.tile([1, QC], bf16, tag="rowb")
                        nc.vector.tensor_copy(grow[:], grow_ps[0:1, :])
                        gb_ps = PS.tile([P, QC], f32, tag="ps", bufs=4)
                        nc.tensor.matmul(gb_ps[:], ones_row[0:1, :], grow[:],
                                         start=True, stop=True)
                        nc.vector.tensor_copy(
                            gbc[:, gh * QC:(gh + 1) * QC], gb_ps[:])

                    return r2bc, gbc, h2q

                def tail_b(ch, x1c, gbc, h2q):
                    ctok = ch * CW
                    # ---- MoE fp8 DoubleRow ----
                    ehq = big.tile([P, 16, 2, CW], f8, tag="ehq")
                    for fc in range(NFC):
                        w1lt = scp.tile([P, 4, 2, P], f8, tag="w1lt")
                        nc.gpsimd.dma_start(w1lt[:], w1l[:, fc].bitcast(f8))
                        pst = [PS.tile([P, QC], f32, tag="ps", bufs=4,
                                       name=f"ps{_t}") for _t in range(2)]
                        for a in range(4):
                            for t in range(2):
                                nc.tensor.matmul(
                                    pst[t][:], w1sb[:, fc, a],
                                    h2q[:, a, :, t * QC:(t + 1) * QC],
                                    start=(a == 0), stop=False,
                                    perf_mode=DR)
                        for a in range(4):
                            for t in range(2):
                                nc.tensor.matmul(
                                    pst[t][:], w1lt[:, a],
                                    h2q[:, a, :, t * QC:(t + 1) * QC],
                                    start=False, stop=(a == 3),
                                    perf_mode=DR)
                        for t in range(2):
                            nc.scalar.activation(
                                ehq[:, fc // 2, fc % 2,
                                    t * QC:(t + 1) * QC],
                                pst[t][:], ACT.Gelu_apprx_tanh,
                                bias=b1_sb[:, fc:fc + 1], scale=1.0 / SC)
                    for dc in range(NDC):
                        w2lt = scp.tile([P, 16, 2, P], f8, tag="w2lt")
                        nc.gpsimd.dma_start(w2lt[:], w2l[:, dc].bitcast(f8))
                        pst = [PS.tile([P, QC], f32, tag="ps", bufs=4,
                                       name=f"ps{_t}") for _t in range(2)]
                        for s in range(16):
                            for t in range(2):
                                nc.tensor.matmul(
                                    pst[t][:], w2sb[:, dc, s],
                                    ehq[:, s, :, t * QC:(t + 1) * QC],
                                    start=(s == 0), stop=False,
                                    perf_mode=DR)
                        for s in range(16):
                            for t in range(2):
                                nc.tensor.matmul(
                                    pst[t][:], w2lt[:, s],
                                    ehq[:, s, :, t * QC:(t + 1) * QC],
                                    start=False, stop=(s == 15),
                                    perf_mode=DR)
                        for t in range(2):
                            tsl = slice(t * QC, (t + 1) * QC)
                            t1 = scp.tile([P, QC], bf16, tag="t1")
                            nc.vector.scalar_tensor_tensor(
                                t1[:], pst[t][:], b2x_sb[:, dc:dc + 1],
                                gbc[:, tsl], op0=ALU.add, op1=ALU.mult)
                            arq = scp.tile([P, QC], bf16, tag="ot", name="arq")
                            nc.sync.dma_start(
                                arq[:], ar_in[ch][dc * P:(dc + 1) * P,
                                                  t * QC:(t + 1) * QC])
                            zt = scp.tile([P, QC], f32, tag="zt")
                            nc.vector.tensor_add(zt[:], arq[:], t1[:])
                            nc.gpsimd.dma_start(
                                z_in[ch][dc * P:(dc + 1) * P,
                                         t * QC:(t + 1) * QC], zt[:])
                    all_reduce(z_in[ch][:], z_out[ch][:])
                    nc.sync.dma_start(outT[:, ctok:ctok + CW], z_out[ch][:])
                    if taps:
                        nc.sync.dma_start(tap_x1[:, ctok:ctok + CW],
                                          ar_out[ch][:])
                        nc.sync.dma_start(tap_z[:, ctok:ctok + CW],
                                          z_in[ch][:])

                for ch in range(NCH):
                    if ch % 2 == 0:
                        qT = ab.tile([P, S], bf16, tag="qT", bufs=1)
                        kT = ab.tile([P, S], bf16, tag="kT", bufs=1)
                        v_aug = ab.tile([P, HPC * NKT * 65], bf16,
                                        tag="v_aug", bufs=1)
                        nc.gpsimd.memset(
                            v_aug[:].rearrange("p (k c) -> p k c", c=65)
                            [:, :, 64:65], 1.0)
                    x1c = attn_phase(ch, qT, kT, v_aug)
                    if ch == 0:
                        nc.sync.dma_start(w1sb[:], w1d[:].bitcast(f8))
                    if ch == 1:
                        nc.sync.dma_start(w2sb[:], w2d[:].bitcast(f8))
                    if ch >= 1:
                        tail_b(ch - 1, *pend.pop(ch - 1))
                    r2bc, gbc, h2q = tail_a(ch, x1c)
                    pend[ch] = (x1c, gbc, h2q)
                tail_b(NCH - 1, *pend.pop(NCH - 1))

    nc.compile()
    _NC_CACHE[key] = nc
    return nc


def make_in_maps(x, n1_w, n2_w, wq, wk, wv, wo, router_w, w1, b1, w2, b2):
    f8np = ml_dtypes.float8_e4m3
    bfnp = ml_dtypes.bfloat16
    x2 = np.asarray(x, np.float32).reshape(T, D)
    xT = np.ascontiguousarray(x2.T)
    xTb = xT.astype(bfnp)
    n1 = np.asarray(n1_w, np.float32)
    n2 = np.asarray(n2_w, np.float32)
    wq_e = (n1[:, None] * np.asarray(wq, np.float32)) * (HD ** -0.5)
    wk_e = n1[:, None] * np.asarray(wk, np.float32)
    wv_e = n1[:, None] * np.asarray(wv, np.float32)
    rw_e = np.ascontiguousarray(
        (np.asarray(router_w, np.float32) * n2[None, :]).T)  # [D, E]
    rw_p = np.ascontiguousarray(
        rw_e.reshape(NDC, P, E).transpose(1, 0, 2)).astype(bfnp)

    def pack_qkv(w):  # [D, HCOL] -> [P, NDC, HCOL]
        return np.ascontiguousarray(
            w.reshape(NDC, P, HCOL).transpose(1, 0, 2)).astype(bfnp)

    in_maps = []
    for c in range(N_CORES):
        cols = slice(c * HCOL, (c + 1) * HCOL)
        w1_e = np.ascontiguousarray(
            ((n2[:, None] * np.asarray(w1[c], np.float32)) * SC)
            .reshape(4, 2, P, NFC, P).transpose(2, 3, 0, 1, 4))   # packed f32
        w1_p = w1_e.astype(f8np)
        w1_r = (w1_e - w1_p.astype(np.float32)).astype(f8np)
        w2_e = np.ascontiguousarray(
            (np.asarray(w2[c], np.float32) * SC)
            .reshape(16, 2, P, NDC, P).transpose(2, 3, 0, 1, 4))
        w2_p = w2_e.astype(f8np)
        w2_r = (w2_e - w2_p.astype(np.float32)).astype(f8np)
        ese = np.zeros((1, E), np.float32)
        ese[0, c] = 1.0 / SC
        in_maps.append({
            "xT": xT,
            "xTb": xTb,
            "wq": pack_qkv(wq_e[:, cols]),
            "wk": pack_qkv(wk_e[:, cols]),
            "wv": pack_qkv(wv_e[:, cols]),
            "wo": np.ascontiguousarray(
                np.asarray(wo, np.float32)[cols, :]).astype(bfnp),
            "rw": rw_p,
            "w1d": w1_p.view(np.uint8),
            "w2d": w2_p.view(np.uint8),
            "w1l": w1_r.view(np.uint8),
            "w2l": w2_r.view(np.uint8),
            "b1": np.ascontiguousarray(
                np.asarray(b1[c], np.float32).reshape(NFC, P)),
            "b2": np.ascontiguousarray(
                np.asarray(b2[c], np.float32).reshape(NDC, P)) * SC,
            "esel": ese.astype(bfnp),
        })
    return in_maps


def kernel(**inputs) -> np.ndarray:
    nc = build_nc()
    in_maps = make_in_maps(**inputs)
    res = run_bass_kernel_spmd(nc, in_maps, core_ids=list(range(N_CORES)),
                               trace=False)
    outT = res.results[0]["outT"]
    return np.ascontiguousarray(outT.T).reshape(B, S, D)


# revision 3
# speedup vs baseline: 1.0157x; 1.0157x over previous
"""Trainium2 Bass kernel for nn_MoEBlock — fp8-DoubleRow MoE, bf16 attention,
chunk-pipelined (attention chunk ch overlaps MoE chunk ch-1 on other engines).

Sharding: heads 2/16 per core (attention), expert e on core e (MoE).
x/8 is injected into the attention AllReduce so ar_out == x1 directly.
z AllReduce (f32) writes straight into outT.
"""

import sys

if "/opt/trn_rl_repo" not in sys.path:
    sys.path.insert(0, "/opt/trn_rl_repo")

import ml_dtypes
import numpy as np

import concourse.bacc as bacc
import concourse.mybir as mybir
import concourse.tile as tile
from concourse.bass_utils import run_bass_kernel_spmd
from concourse.masks import make_identity

B, S, D, H, F, E, K = 2, 2048, 1024, 16, 4096, 8, 2
HD = D // H          # 64
T = B * S            # 4096
EPS = 1e-6
N_CORES = 8
HPC = H // N_CORES   # 2 heads per core
HCOL = HPC * HD      # 128

P = 128
QC = 512             # token tile (matmul free dim)
CW = 1024            # chunk width (tokens); 4 chunks
NCH = T // CW        # 4
NKT = S // P         # 16 k-tiles per batch
NDC = D // P         # 8
NFC = F // P         # 32
SC = 32.0            # fp8 weight scale

f32 = mybir.dt.float32
f32r = mybir.dt.float32r
bf16 = mybir.dt.bfloat16
f8 = mybir.dt.float8e4
AX = mybir.AxisListType
ALU = mybir.AluOpType
ACT = mybir.ActivationFunctionType
DR = mybir.MatmulPerfMode.DoubleRow

_NC_CACHE = {}


def build_nc(sim_mode=False, taps=False):
    key = (sim_mode, taps)
    if key in _NC_CACHE:
        return _NC_CACHE[key]
    nc = bacc.Bacc("TRN2", target_bir_lowering=False, debug=False,
                   num_devices=1 if sim_mode else N_CORES)

    def all_reduce(src_ap, dst_ap):
        if sim_mode:
            nc.sync.dma_start(dst_ap[0:1, :], src_ap[0:1, :])
        else:
            nc.gpsimd.collective_compute(
                "AllReduce", ALU.add,
                replica_groups=[list(range(N_CORES))],
                ins=[src_ap], outs=[dst_ap],
            )

    # ---- I/O ----
    xT = nc.dram_tensor("xT", [D, T], f32, kind="ExternalInput")
    xTb = nc.dram_tensor("xTb", [D, T], bf16, kind="ExternalInput")
    wq = nc.dram_tensor("wq", [P, NDC, HCOL], bf16, kind="ExternalInput")
    wk = nc.dram_tensor("wk", [P, NDC, HCOL], bf16, kind="ExternalInput")
    wv = nc.dram_tensor("wv", [P, NDC, HCOL], bf16, kind="ExternalInput")
    wo = nc.dram_tensor("wo", [HCOL, D], bf16, kind="ExternalInput")
    rw = nc.dram_tensor("rw", [P, NDC, E], bf16, kind="ExternalInput")
    w1d = nc.dram_tensor("w1d", [P, NFC, 4, 2, P], mybir.dt.uint8,
                         kind="ExternalInput")
    w2d = nc.dram_tensor("w2d", [P, NDC, 16, 2, P], mybir.dt.uint8,
                         kind="ExternalInput")
    w1l = nc.dram_tensor("w1l", [P, NFC, 4, 2, P], mybir.dt.uint8,
                         kind="ExternalInput")
    w2l = nc.dram_tensor("w2l", [P, NDC, 16, 2, P], mybir.dt.uint8,
                         kind="ExternalInput")
    b1 = nc.dram_tensor("b1", [NFC, P], f32, kind="ExternalInput")
    b2 = nc.dram_tensor("b2", [NDC, P], f32, kind="ExternalInput")
    esel = nc.dram_tensor("esel", [1, E], bf16, kind="ExternalInput")
    outT = nc.dram_tensor("outT", [D, T], f32, kind="ExternalOutput")
    if taps:
        tap_x1 = nc.dram_tensor("tap_x1", [D, T], bf16, kind="ExternalOutput")
        tap_z = nc.dram_tensor("tap_z", [D, T], f32, kind="ExternalOutput")

    with tile.TileContext(nc) as tc:
        with (
            tc.tile_pool(name="const", bufs=1) as cp,
            tc.tile_pool(name="dram", bufs=1, space="DRAM") as dp,
        ):
            # ---- constants / weights (resident) ----
            ident = cp.tile([P, P], f32, tag="ident")
            make_identity(nc, ident[:])
            ident_bf = cp.tile([P, P], bf16, tag="ident_bf")
            nc.vector.tensor_copy(ident_bf[:], ident[:])
            ones_bf = cp.tile([P, 1], bf16, tag="ones_bf")
            nc.gpsimd.memset(ones_bf[:], 1.0)
            ones_f8 = cp.tile([P, 1], f8, tag="ones_f8")
            nc.gpsimd.memset(ones_f8[:], 1.0)
            ones_row = cp.tile([1, P], bf16, tag="ones_row")
            nc.gpsimd.memset(ones_row[:], 1.0)
            eps_col = cp.tile([P, 1], f32, tag="eps_col")
            nc.gpsimd.memset(eps_col[:], EPS)
            masks = cp.tile([P, QC + 3 * P], bf16, tag="masks")
            nc.gpsimd.memset(masks[:], 1.0)
            nc.gpsimd.affine_select(
                out=masks[:], in_=masks[:],
                compare_op=ALU.is_ge, fill=0.0, base=-3 * P,
                pattern=[[1, QC + 3 * P]], channel_multiplier=-1,
            )
            b1_sb = cp.tile([P, NFC], f32, tag="b1_sb")
            nc.sync.dma_start(b1_sb[:], b1[:].rearrange("a p -> p a"))
            b2x_sb = cp.tile([P, NDC], f32, tag="b2x_sb")
            nc.sync.dma_start(b2x_sb[:], b2[:].rearrange("a p -> p a"))
            esel_bc = cp.tile([P, E], bf16, tag="esel_bc")
            nc.sync.dma_start(esel_bc[:], esel[0:1, :].to_broadcast((P, E)))

            wq_sb = cp.tile([P, NDC, HCOL], bf16, tag="wq_sb")
            wk_sb = cp.tile([P, NDC, HCOL], bf16, tag="wk_sb")
            wv_sb = cp.tile([P, NDC, HCOL], bf16, tag="wv_sb")
            wo_sb = cp.tile([P, D], bf16, tag="wo_sb")
            rw_sb = cp.tile([P, NDC, E], bf16, tag="rw_sb")
            for w_sb, w_dr in ((wq_sb, wq), (wk_sb, wk), (wv_sb, wv)):
                nc.sync.dma_start(w_sb[:], w_dr[:])
            nc.sync.dma_start(wo_sb[:], wo[:])
            nc.sync.dma_start(rw_sb[:], rw[:])
            w1sb = cp.tile([P, NFC, 4, 2, P], f8, tag="w1sb")
            w2sb = cp.tile([P, NDC, 16, 2, P], f8, tag="w2sb")

            # ---- DRAM scratch ----
            ar_in = [dp.tile([D, CW], bf16, tag=f"ar_in{i}", name=f"ar_in{i}")
                     for i in range(NCH)]
            ar_out = [dp.tile([D, CW], bf16, tag=f"ar_out{i}", name=f"ar_out{i}",
                              addr_space="Shared") for i in range(NCH)]
            z_in = [dp.tile([D, CW], f32, tag=f"z_in{i}", name=f"z_in{i}")
                    for i in range(NCH)]
            z_out = [dp.tile([D, CW], f32, tag=f"z_out{i}", name=f"z_out{i}",
                             addr_space="Shared") for i in range(NCH)]

            with (
                # psum pools
                tc.tile_pool(name="PS", bufs=1, space="PSUM") as PS,
                # persistent/rotating sbuf pools
                tc.tile_pool(name="ab", bufs=1) as ab,     # per-batch attn
                tc.tile_pool(name="sc", bufs=2) as scp,    # rotating scratch
                tc.tile_pool(name="big", bufs=1) as big,   # single-buffer big
                tc.tile_pool(name="rows", bufs=2) as rp,   # [1,*] rows
            ):
                qT = kT = v_aug = None
                pend = {}

                def attn_phase(ch, qT, kT, v_aug):
                    b_ = ch // 2
                    hf = ch % 2
                    ctok = ch * CW

                    # ---- projections (2 x 512 tokens) ----
                    for tch in range(2):
                        tok = ctok + tch * QC
                        tsl = slice(tok, tok + QC)
                        csl = slice(hf * CW + tch * QC,
                                    hf * CW + (tch + 1) * QC)
                        q_ps = PS.tile([P, QC], f32, tag="ps", bufs=4)
                        k_ps = PS.tile([P, QC], f32, tag="ps", bufs=4)
                        v_ps = PS.tile([P, QC], f32, tag="ps", bufs=4)
                        ssp = PS.tile([P, QC], f32, tag="ps", bufs=4)
                        for h4 in range(2):
                            xtb = scp.tile([P, 4, QC], bf16, tag="xtb")
                            nc.scalar.dma_start(
                                xtb[:],
                                xTb[4 * h4 * P:(4 * h4 + 4) * P, tsl]
                                .rearrange("(a p) t -> p a t", p=P))
                            sq = scp.tile([P, 4, QC], f8, tag="sq")
                            nc.scalar.activation(sq[:], xtb[:], ACT.Square)
                            for d4 in range(4):
                                dc = 4 * h4 + d4
                                st = (dc == 0)
                                sp = (dc == NDC - 1)
                                xd = xtb[:, d4, :]
                                nc.tensor.matmul(q_ps[:], wq_sb[:, dc, :], xd,
                                                 start=st, stop=sp)
                                nc.tensor.matmul(k_ps[:], wk_sb[:, dc, :], xd,
                                                 start=st, stop=sp)
                                nc.tensor.matmul(v_ps[:], wv_sb[:, dc, :], xd,
                                                 start=st, stop=sp)
                                nc.tensor.matmul(
                                    ssp[0:1, :], ones_f8[:, 0:1], sq[:, d4, :],
                                    start=st, stop=sp)
                        msr = rp.tile([1, QC], f32, tag="rowf")
                        nc.vector.tensor_scalar(msr[:], ssp[0:1, :], 1.0 / D,
                                                EPS, op0=ALU.mult, op1=ALU.add)
                        srr = rp.tile([1, QC], f32, tag="rowf")
                        nc.scalar.sqrt(srr[:], msr[:])
                        r1r = rp.tile([1, QC], bf16, tag="rowb")
                        with nc.allow_low_precision(reason="bf16 rsqrt"):
                            nc.vector.reciprocal(r1r[:], srr[:])
                        r1bp = PS.tile([P, QC], f32, tag="ps", bufs=4)
                        nc.tensor.matmul(r1bp[:], ones_row[0:1, :], r1r[:],
                                         start=True, stop=True)
                        r1s = scp.tile([P, QC], bf16, tag="r1s")
                        nc.vector.tensor_copy(r1s[:], r1bp[:])
                        nc.vector.tensor_mul(qT[:, csl], q_ps[:], r1s[:])
                        nc.vector.tensor_mul(kT[:, csl], k_ps[:], r1s[:])
                        vts = scp.tile([P, QC], bf16, tag="vts", bufs=1)
                        nc.vector.tensor_mul(vts[:], v_ps[:], r1s[:])
                        for blk in range(QC // P):
                            kt_ = hf * 8 + tch * 4 + blk
                            vtp = PS.tile([P, P], bf16, tag="tpb", bufs=1)
                            nc.tensor.transpose(
                                vtp[:], vts[:, blk * P:(blk + 1) * P],
                                ident_bf[:])
                            for h in range(HPC):
                                idx = (h * NKT + kt_) * 65
                                nc.vector.tensor_copy(
                                    v_aug[:, idx:idx + HD],
                                    vtp[:, h * HD:(h + 1) * HD])

                    # ---- scores / softmax / ctx ----
                    ctxn = big.tile([P, CW], bf16, tag="ctxn")
                    for qcl in range(2):
                        qg = hf * 2 + qcl
                        qsl = slice(qg * QC, (qg + 1) * QC)
                        nkt = (qg + 1) * (QC // P)
                        for h in range(HPC):
                            hsl = slice(h * HD, (h + 1) * HD)
                            cps = PS.tile([65, QC], f32, tag="cps", bufs=2)
                            for kt_ in range(nkt):
                                ksl = slice(kt_ * P, (kt_ + 1) * P)
                                sps = PS.tile([P, QC], f32, tag="ps", bufs=4)
                                nc.tensor.matmul(
                                    sps[:], kT[hsl, ksl], qT[hsl, qsl],
                                    start=True, stop=True)
                                ex = scp.tile([P, QC], bf16, tag="ex", bufs=2)
                                nc.scalar.activation(ex[:], sps[:], ACT.Exp)
                                j = kt_ - qg * (QC // P)
                                if j >= 0:
                                    nc.vector.tensor_mul(
                                        ex[:], ex[:],
                                        masks[:, (3 - j) * P:
                                              (3 - j) * P + QC])
                                idx = (h * NKT + kt_) * 65
                                nc.tensor.matmul(
                                    cps[:], v_aug[:, idx:idx + 65], ex[:],
                                    start=(kt_ == 0), stop=(kt_ == nkt - 1))
                            rec = rp.tile([1, QC], bf16, tag="rowb")
                            with nc.allow_low_precision(reason="softmax recip"):
                                nc.vector.reciprocal(rec[:], cps[64:65, :])
                            bc = PS.tile([P, QC], f32, tag="ps", bufs=4)
                            nc.tensor.matmul(bc[0:HD, :],
                                             ones_row[0:1, 0:HD], rec[:],
                                             start=True, stop=True)
                            bcs = scp.tile([HD, QC], bf16, tag="bcs")
                            nc.vector.tensor_copy(bcs[:], bc[0:HD, :])
                            nc.vector.tensor_mul(
                                ctxn[hsl, qcl * QC:(qcl + 1) * QC],
                                cps[0:HD, :], bcs[:])

                    # ---- wo + x/8 inject + AR ----
                    for t in range(2):
                        tsl = slice(ctok + t * QC, ctok + (t + 1) * QC)
                        for dc in range(NDC):
                            wop = PS.tile([P, QC], f32, tag="ps", bufs=4)
                            nc.tensor.matmul(
                                wop[:], wo_sb[:, dc * P:(dc + 1) * P],
                                ctxn[:, t * QC:(t + 1) * QC],
                                start=True, stop=True)
                            xq = scp.tile([P, QC], f32, tag="xq")
                            nc.sync.dma_start(
                                xq[:], xT[dc * P:(dc + 1) * P, tsl])
                            ot = scp.tile([P, QC], bf16, tag="ot")
                            nc.vector.scalar_tensor_tensor(
                                ot[:], xq[:], 1.0 / N_CORES, wop[:],
                                op0=ALU.mult, op1=ALU.add)
                            nc.sync.dma_start(
                                ar_in[ch][dc * P:(dc + 1) * P,
                                          t * QC:(t + 1) * QC], ot[:])
                    all_reduce(ar_in[ch][:], ar_out[ch][:])
                    x1c = []
                    for t in range(2):
                        x1h = big.tile([P, NDC, QC], bf16, tag="x1c", bufs=2,
                                       name=f"x1h{t}")
                        nc.sync.dma_start(
                            x1h[:],
                            ar_out[ch][:, t * QC:(t + 1) * QC].rearrange(
                                "(a p) t -> p a t", p=P))
                        x1c.append(x1h)
                    return x1c

                def tail_a(ch, x1c):
                    ctok = ch * CW
                    # ---- r2, router logits ----
                    lgc = scp.tile([E, CW], f32, tag="lgc", bufs=1)
                    r2bc = scp.tile([P, CW], bf16, tag="r2bc", bufs=1)
                    for t in range(2):
                        tsl = slice(t * QC, (t + 1) * QC)
                        ss2 = PS.tile([P, QC], f32, tag="ps", bufs=4)
                        lgp = PS.tile([P, QC], f32, tag="ps", bufs=4)
                        sq2 = None
                        for dc in range(NDC):
                            st = (dc == 0)
                            sp = (dc == NDC - 1)
                            if dc % 4 == 0:
                                sq2 = scp.tile([P, 4, QC], f8, tag="sq",
                                               bufs=2, name="sq2")
                                nc.scalar.activation(
                                    sq2[:], x1c[t][:, dc:dc + 4, :],
                                    ACT.Square)
                            nc.tensor.matmul(
                                ss2[0:1, :], ones_f8[:, 0:1],
                                sq2[:, dc % 4, :], start=st, stop=sp)
                            nc.tensor.matmul(
                                lgp[0:E, :], rw_sb[:, dc, :],
                                x1c[t][:, dc, :], start=st, stop=sp)
                        nc.scalar.activation(lgc[:, tsl], lgp[0:E, :],
                                             ACT.Copy)
                        msr2 = rp.tile([1, QC], f32, tag="rowf")
                        nc.vector.tensor_scalar(msr2[:], ss2[0:1, :], 1.0 / D,
                                                EPS, op0=ALU.mult, op1=ALU.add)
                        srr2 = rp.tile([1, QC], f32, tag="rowf")
                        nc.scalar.sqrt(srr2[:], msr2[:])
                        r2r = rp.tile([1, QC], bf16, tag="rowb")
                        with nc.allow_low_precision(reason="bf16 rsqrt"):
                            nc.vector.reciprocal(r2r[:], srr2[:])
                        r2bp = PS.tile([P, QC], f32, tag="ps", bufs=4)
                        nc.tensor.matmul(r2bp[:], ones_row[0:1, :], r2r[:],
                                         start=True, stop=True)
                        nc.scalar.activation(
                            r2bc[:, t * QC:(t + 1) * QC], r2bp[:], ACT.Copy)

                    h2q = big.tile([P, 4, 2, CW], f8, tag="h2q", bufs=1)
                    for t in range(2):
                        for dc in range(NDC):
                            nc.vector.tensor_mul(
                                h2q[:, dc // 2, dc % 2,
                                    t * QC:(t + 1) * QC],
                                x1c[t][:, dc, :],
                                r2bc[:, t * QC:(t + 1) * QC])
                    # ---- r2 per token-tile columns (negated) ----
                    tpg = PS.tile([P, CW // P * E], f32, tag="tps", bufs=1)
                    for i in range(CW // P):
                        nc.tensor.matmul(
                            tpg[:, i:i + 1],
                            r2bc[0:1, i * P:(i + 1) * P], ones_bf[0:1, 0:1],
                            start=True, stop=True)
                    r2col = scp.tile([P, E], f32, tag="r2col", bufs=1)
                    nc.vector.tensor_copy(r2col[:], tpg[:, 0:E])

                    # ---- gates (per token-tile, [P,1]-scalar ops) ----
                    ltp = PS.tile([P, CW // P * E], f32, tag="tps", bufs=1)
                    for tt in range(CW // P):
                        nc.tensor.transpose(
                            ltp[:, tt * E:(tt + 1) * E],
                            lgc[:, tt * P:(tt + 1) * P], ident[0:E, 0:E])
                    gcols = scp.tile([P, CW // P], bf16, tag="gcols", bufs=1)
                    for tt in range(CW // P):
                        lg = scp.tile([P, E], f32, tag="lg", bufs=3)
                        nc.vector.tensor_copy(lg[:], ltp[:, tt * E:(tt + 1) * E])
                        m1 = scp.tile([P, 1], f32, tag="m1", bufs=3)
                        nc.vector.tensor_reduce(m1[:], lg[:], axis=AX.X,
                                                op=ALU.max)
                        mk1 = scp.tile([P, E], f32, tag="mk1", bufs=3)
                        nc.vector.tensor_scalar(mk1[:], lg[:], m1[:], None,
                                                op0=ALU.is_equal)
                        msk = scp.tile([P, E], f32, tag="msk", bufs=3)
                        nc.vector.scalar_tensor_tensor(
                            msk[:], mk1[:], -1e30, lg[:],
                            op0=ALU.mult, op1=ALU.add)
                        m2 = scp.tile([P, 1], f32, tag="m2", bufs=3)
                        nc.vector.tensor_reduce(m2[:], msk[:], axis=AX.X,
                                                op=ALU.max)
                        mk2 = scp.tile([P, E], f32, tag="mk2", bufs=3)
                        nc.vector.tensor_scalar(mk2[:], msk[:], m2[:], None,
                                                op0=ALU.is_equal)
                        dlt = scp.tile([P, 1], f32, tag="dlt", bufs=3)
                        nc.vector.tensor_sub(dlt[:], m1[:], m2[:])
                        g1 = scp.tile([P, 1], f32, tag="g1", bufs=3)
                        nc.scalar.activation(g1[:], dlt[:], ACT.Sigmoid,
                                             scale=r2col[:, tt:tt + 1])
                        g2_ = scp.tile([P, 1], f32, tag="g2_", bufs=3)
                        nc.vector.tensor_scalar(g2_[:], g1[:], -1.0, 1.0,
                                                op0=ALU.mult, op1=ALU.add)
                        gts = scp.tile([P, E], f32, tag="gts", bufs=3)
                        nc.vector.tensor_scalar(gts[:], mk1[:], g1[:], None,
                                                op0=ALU.mult)
                        nc.vector.scalar_tensor_tensor(
                            gts[:], mk2[:], g2_[:], gts[:],
                            op0=ALU.mult, op1=ALU.add)
                        gsel = scp.tile([P, E], bf16, tag="gsel", bufs=3)
                        nc.vector.tensor_mul(gsel[:], gts[:], esel_bc[:])
                        with nc.allow_low_precision(reason="bf16 gate col"):
                            nc.vector.tensor_reduce(gcols[:, tt:tt + 1],
                                                    gsel[:], axis=AX.X,
                                                    op=ALU.add)
                    gbc = scp.tile([P, CW], bf16, tag="gbc", bufs=1)
                    for gh in range(2):
                        grow_ps = PS.tile([P, QC], f32, tag="ps", bufs=4)
                        for tq in range(QC // P):
                            tt = gh * (QC // P) + tq
                            nc.tensor.matmul(
                                grow_ps[0:1, tq * P:(tq + 1) * P],
                                gcols[:, tt:tt + 1], ident_bf[:],
                                start=True, stop=True)
                        grow = rp.tile([1, QC], bf16, tag="rowb")
                        nc.vector.tensor_copy(grow[:], grow_ps[0:1, :])
                        gb_ps = PS.tile([P, QC], f32, tag="ps", bufs=4)
                        nc.tensor.matmul(gb_ps[:], ones_row[0:1, :], grow[:],
                                         start=True, stop=True)
                        nc.scalar.activation(
                            gbc[:, gh * QC:(gh + 1) * QC], gb_ps[:],
                            ACT.Copy)

                    return r2bc, gbc, h2q

                def tail_b(ch, x1c, gbc, h2q):
                    ctok = ch * CW
                    # ---- MoE fp8 DoubleRow ----
                    ehq = big.tile([P, 16, 2, CW], f8, tag="ehq")
                    for fc in range(NFC):
                        w1lt = scp.tile([P, 4, 2, P], f8, tag="w1lt")
                        nc.gpsimd.dma_start(w1lt[:], w1l[:, fc].bitcast(f8))
                        pst = [PS.tile([P, QC], f32, tag="ps", bufs=4,
                                       name=f"ps{_t}") for _t in range(2)]
                        for a in range(4):
                            for t in range(2):
                                nc.tensor.matmul(
                                    pst[t][:], w1sb[:, fc, a],
                                    h2q[:, a, :, t * QC:(t + 1) * QC],
                                    start=(a == 0), stop=False,
                                    perf_mode=DR)
                        for a in range(4):
                            for t in range(2):
                                nc.tensor.matmul(
                                    pst[t][:], w1lt[:, a],
                                    h2q[:, a, :, t * QC:(t + 1) * QC],
                                    start=False, stop=(a == 3),
                                    perf_mode=DR)
                        for t in range(2):
                            nc.scalar.activation(
                                ehq[:, fc // 2, fc % 2,
                                    t * QC:(t + 1) * QC],
                                pst[t][:], ACT.Gelu_apprx_tanh,
                                bias=b1_sb[:, fc:fc + 1], scale=1.0 / SC)
                    for dc in range(NDC):
                        w2lt = scp.tile([P, 16, 2, P], f8, tag="w2lt")
                        nc.gpsimd.dma_start(w2lt[:], w2l[:, dc].bitcast(f8))
                        pst = [PS.tile([P, QC], f32, tag="ps", bufs=4,
                                       name=f"ps{_t}") for _t in range(2)]
                        for s in range(16):
                            for t in range(2):
                                nc.tensor.matmul(
                                    pst[t][:], w2sb[:, dc, s],
                                    ehq[:, s, :, t * QC:(t + 1) * QC],
                                    start=(s == 0), stop=False,
                                    perf_mode=DR)
                        for s in range(16):
                            for t in range(2):
                                nc.tensor.matmul(
                                    pst[t][:], w2lt[:, s],
                                    ehq[:, s, :, t * QC:(t + 1) * QC],
                                    start=False, stop=(s == 15),
                                    perf_mode=DR)
                        for t in range(2):
                            tsl = slice(t * QC, (t + 1) * QC)
                            t1 = scp.tile([P, QC], bf16, tag="t1")
                            nc.vector.scalar_tensor_tensor(
                                t1[:], pst[t][:], b2x_sb[:, dc:dc + 1],
                                gbc[:, tsl], op0=ALU.add, op1=ALU.mult)
                            arq = scp.tile([P, QC], bf16, tag="ot", name="arq")
                            nc.sync.dma_start(
                                arq[:], ar_in[ch][dc * P:(dc + 1) * P,
                                                  t * QC:(t + 1) * QC])
                            zt = scp.tile([P, QC], f32, tag="zt")
                            nc.vector.tensor_add(zt[:], arq[:], t1[:])
                            nc.gpsimd.dma_start(
                                z_in[ch][dc * P:(dc + 1) * P,
                                         t * QC:(t + 1) * QC], zt[:])
                    all_reduce(z_in[ch][:], z_out[ch][:])
                    nc.sync.dma_start(outT[:, ctok:ctok + CW], z_out[ch][:])
                    if taps:
                        nc.sync.dma_start(tap_x1[:, ctok:ctok + CW],
                                          ar_out[ch][:])
                        nc.sync.dma_start(tap_z[:, ctok:ctok + CW],
                                          z_in[ch][:])

                for ch in range(NCH):
                    if ch % 2 == 0:
                        qT = ab.tile([P, S], bf16, tag="qT", bufs=1)
                        kT = ab.tile([P, S], bf16, tag="kT", bufs=1)
                        v_aug = ab.tile([P, HPC * NKT * 65], bf16,
                                        tag="v_aug", bufs=1)
                        nc.gpsimd.memset(
                            v_aug[:].rearrange("p (k c) -> p k c", c=65)
                            [:, :, 64:65], 1.0)
                    x1c = attn_phase(ch, qT, kT, v_aug)
                    if ch == 1:
                        nc.sync.dma_start(w1sb[:], w1d[:].bitcast(f8))
                        nc.sync.dma_start(w2sb[:], w2d[:].bitcast(f8))
                    if ch >= 1:
                        tail_b(ch - 1, *pend.pop(ch - 1))
                    r2bc, gbc, h2q = tail_a(ch, x1c)
                    pend[ch] = (x1c, gbc, h2q)
                tail_b(NCH - 1, *pend.pop(NCH - 1))

    nc.compile()
    _NC_CACHE[key] = nc
    return nc


def make_in_maps(x, n1_w, n2_w, wq, wk, wv, wo, router_w, w1, b1, w2, b2):
    f8np = ml_dtypes.float8_e4m3
    bfnp = ml_dtypes.bfloat16
    x2 = np.asarray(x, np.float32).reshape(T, D)
    xT = np.ascontiguousarray(x2.T)
    xTb = xT.astype(bfnp)
    n1 = np.asarray(n1_w, np.float32)
    n2 = np.asarray(n2_w, np.float32)
    wq_e = (n1[:, None] * np.asarray(wq, np.float32)) * (HD ** -0.5)
    wk_e = n1[:, None] * np.asarray(wk, np.float32)
    wv_e = n1[:, None] * np.asarray(wv, np.float32)
    rw_e = np.ascontiguousarray(
        (np.asarray(router_w, np.float32) * n2[None, :]).T)  # [D, E]
    rw_p = np.ascontiguousarray(
        rw_e.reshape(NDC, P, E).transpose(1, 0, 2)).astype(bfnp)

    def pack_qkv(w):  # [D, HCOL] -> [P, NDC, HCOL]
        return np.ascontiguousarray(
            w.reshape(NDC, P, HCOL).transpose(1, 0, 2)).astype(bfnp)

    in_maps = []
    for c in range(N_CORES):
        cols = slice(c * HCOL, (c + 1) * HCOL)
        w1_e = np.ascontiguousarray(
            ((n2[:, None] * np.asarray(w1[c], np.float32)) * SC)
            .reshape(4, 2, P, NFC, P).transpose(2, 3, 0, 1, 4))   # packed f32
        w1_p = w1_e.astype(f8np)
        w1_r = (w1_e - w1_p.astype(np.float32)).astype(f8np)
        w2_e = np.ascontiguousarray(
            (np.asarray(w2[c], np.float32) * SC)
            .reshape(16, 2, P, NDC, P).transpose(2, 3, 0, 1, 4))
        w2_p = w2_e.astype(f8np)
        w2_r = (w2_e - w2_p.astype(np.float32)).astype(f8np)
        ese = np.zeros((1, E), np.float32)
        ese[0, c] = 1.0 / SC
        in_maps.append({
            "xT": xT,
            "xTb": xTb,
            "wq": pack_qkv(wq_e[:, cols]),
            "wk": pack_qkv(wk_e[:, cols]),
            "wv": pack_qkv(wv_e[:, cols]),
            "wo": np.ascontiguousarray(
                np.asarray(wo, np.float32)[cols, :]).astype(bfnp),
            "rw": rw_p,
            "w1d": w1_p.view(np.uint8),
            "w2d": w2_p.view(np.uint8),
            "w1l": w1_r.view(np.uint8),
            "w2l": w2_r.view(np.uint8),
            "b1": np.ascontiguousarray(
                np.asarray(b1[c], np.float32).reshape(NFC, P)),
            "b2": np.ascontiguousarray(
                np.asarray(b2[c], np.float32).reshape(NDC, P)) * SC,
            "esel": ese.astype(bfnp),
        })
    return in_maps


def kernel(**inputs) -> np.ndarray:
    nc = build_nc()
    in_maps = make_in_maps(**inputs)
    res = run_bass_kernel_spmd(nc, in_maps, core_ids=list(range(N_CORES)),
                               trace=False)
    outT = res.results[0]["outT"]
    return np.ascontiguousarray(outT.T).reshape(B, S, D)
